# revision 22
# baseline (speedup 1.0000x reference)
"""Trainium2 Bass kernel for nn_ActorNetwork (GNN message passing actor).

Self-contained: takes full unsharded inputs, shards across 8 NeuronCores
internally, runs one SPMD Bass/Tile kernel, returns the full [N, 8] output.

Structure (per core, 1250 nodes each, edges assigned to dst's owner):
  - node/glob embedders + per-node projections -> fp16 table [1280, 512]
  - AllGather the table across the 8 cores
  - per-edge dma_gather of the two endpoint projections, edge-attr MLP,
    fused edge update, segment-mean via one-hot matmuls into PSUM
  - node update + head (with folded weights), tanh, DMA out

Matmuls run in float32r (full PE rate at N>=256); tables / edge path in
fp16. Dead code in the reference (global-state update) is skipped: the
output depends only on the node path.
"""

import sys
import types

import numpy as np


def _install_ntff_hook():
    if "antenv.axon_hooks" in sys.modules:
        return
    try:
        from trn_agent_boot.trn_boot import _ntff_profile_via_ctypes
        hook = _ntff_profile_via_ctypes("/opt/axon/libaxon_pjrt.so")
    except Exception:
        hook = None
    mod = types.ModuleType("antenv.axon_hooks")
    mod.get_axon_ntff_profile_hook = lambda: hook
    mod.set_axon_ntff_profile_hook = lambda h: None
    sys.modules["antenv.axon_hooks"] = mod


_install_ntff_hook()

import concourse.bacc as bacc
import concourse.mybir as mybir
import concourse.tile as tile
from concourse.bass_utils import run_bass_kernel_spmd
from concourse.masks import make_identity

F32 = mybir.dt.float32
F32R = mybir.dt.float32r
F16 = mybir.dt.float16
I16 = mybir.dt.int16
RELU = mybir.ActivationFunctionType.Relu
COPY = mybir.ActivationFunctionType.Copy
TANH = mybir.ActivationFunctionType.Tanh

N, E, G, CORES = 10000, 160000, 256, 8
NPC = N // CORES            # 1250 nodes per core
NPAD = 1280                 # padded to 10 windows of 128
NW = NPAD // 128            # node windows per core
TBL_ROWS = CORES * NPAD     # AllGather table rows


def _split_excess_waits(nc, max_waits=1):
    for bb in nc.main_func.blocks:
        out_list = []
        for ins in bb.instructions:
            si = ins.sync_info
            if si is not None and si.on_wait is not None and len(si.on_wait) > max_waits:
                waits = list(si.on_wait)
                keep = waits[-max_waits:]
                extra = waits[:-max_waits]
                for i in range(0, len(extra), max_waits):
                    chunk = extra[i:i + max_waits]
                    d = mybir.InstDrain(
                        name=nc.get_next_instruction_name(),
                        ins=[], outs=[], bass_is_fusable=False,
                    )
                    d.engine = ins.engine
                    d.sync_info = mybir.SyncInfo(on_wait=list(chunk), on_update=[])
                    out_list.append(d)
                si.on_wait = keep
            out_list.append(ins)
        bb.instructions[:] = out_list


def _wrap_idx(idx_flat):
    """int16 index array -> [128, ceil(n/16)] wrapped+replicated layout."""
    n = len(idx_flat)
    cols = (n + 15) // 16
    w = np.zeros((16, cols), dtype=np.int16)
    w[np.arange(n) % 16, np.arange(n) // 16] = idx_flat
    return np.tile(w, (8, 1))


def _build_graph(win_tiles, split_waits=True, phases=3):
    """Build the SPMD Bacc graph. win_tiles[w] = edge tiles in window w."""
    win_tiles = [int(t) for t in win_tiles]
    EPC = sum(win_tiles) * 128          # padded edges per core
    ecols = EPC // 16
    TWMAX = max(win_tiles)

    nc = bacc.Bacc("TRN2", target_bir_lowering=False, debug=False,
                   num_devices=CORES)

    dp = lambda name, shape, dt: nc.declare_dram_parameter(
        name, list(shape), dt, isOutput=False)

    # --- per-core shards ---
    xT = dp("xT", [32, NPAD], F32)
    uT = dp("uT", [32, G], F32)
    eaT = dp("eaT", [16, EPC], F16)
    sidx = dp("sidx", [128, ecols], I16)
    mmat = dp("mmat", [EPC // 128, 128, 128], F16)
    mmatT = dp("mmatT", [EPC // 128, 128, 128], F16)
    smat = dp("smat", [128, 2 * NPAD], F16)
    mrow = dp("mrow", [1, NPAD], F32)

    # --- replicated weights ---
    wnames = {}
    for nm, shape in [
        ("w1n", [32, 512]), ("b1n", [512]), ("w2n", [512, 512]), ("b2n", [512]),
        ("w3n", [512, 512]), ("b3n", [512]),
        ("w1e", [16, 256]), ("b1e", [256]), ("w2e", [256, 256]), ("b2e", [256]),
        ("we3", [256, 256]), ("be3", [256]),
        ("w1g", [32, 512]), ("b1g", [512]), ("w2g", [512, 512]), ("b2g", [512]),
        ("w3g", [512, 512]), ("b3g", [512]),
        ("w1m", [1792, 256]), ("b1m", [256]), ("w2m", [256, 256]), ("b2m", [256]),
        ("wn1", [1280, 256]), ("bn1", [256]), ("wn2", [256, 512]), ("bn2", [512]),
        ("wh1", [512, 256]), ("bh1", [256]), ("wh2", [256, 8]), ("bh2", [8]),
    ]:
        wnames[nm] = dp(nm, shape, F32)
    W = wnames

    out = nc.declare_dram_parameter("out", [8, NPAD], F32, isOutput=True)

    ag_in = nc.dram_tensor("ag_in", [NPAD, 256], F16)
    ag_out = nc.dram_tensor("ag_out", [TBL_ROWS, 256], F16, addr_space="Shared")

    kview = lambda w: w[:].rearrange("(kt p) m -> p kt m", p=128)
    rowslice = lambda w, lo, hi: w[lo:hi, :].rearrange("(kt p) m -> p kt m", p=128)
    bview = lambda b: b[:].rearrange("(t p) -> p t", p=128)

    with tile.TileContext(nc, num_cores=CORES) as tc:
        with (
            tc.tile_pool(name="wpool", bufs=1) as wp,
            tc.tile_pool(name="apool", bufs=1) as apool,
            tc.tile_pool(name="ps_wide", bufs=3, space="PSUM") as ps_wide,
            tc.tile_pool(name="ps_tok", bufs=3, space="PSUM") as ps_tok,
            tc.tile_pool(name="ps_agg", bufs=2, space="PSUM") as ps_agg,
        ):
            pw = lambda: ps_wide.tile([128, 512], F32, tag="wide", name="pwide")
            pk = lambda shape, dt=F32: ps_tok.tile(shape, dt, tag="tok", name="ptok")

            # identities / ones (persistent, small)
            identf = wp.tile([128, 128], F32, tag="identf")
            make_identity(nc, identf[:])
            ident16 = wp.tile([128, 128], F16, tag="ident16")
            nc.vector.tensor_copy(ident16[:], identf[:])
            identr = wp.tile([128, 128], F32R, tag="identr")
            nc.vector.tensor_copy(identr[:], identf[:])
            ones_f = wp.tile([1, 512], F32, tag="ones_f")
            nc.gpsimd.memset(ones_f[:], 1.0)
            zero_f = wp.tile([128, 512], F32, tag="zero_f")
            nc.gpsimd.memset(zero_f[:], 0.0)
            ones_row = wp.tile([1, 512], F32R, tag="ones_row")
            nc.vector.tensor_copy(ones_row[:], ones_f[:])

            # persistent activations
            h_fm = apool.tile([128, 4, NPAD], F32R, tag="h_fm")
            agg_fm = apool.tile([128, 2, NPAD], F32R, tag="agg_fm")
            g_fm = apool.tile([128, 4, G], F32R, tag="g_fm")
            gproj_pad = apool.tile([128, 2, 512], F16, tag="gproj_pad")
            gproj2_tm = apool.tile([128, 2, 256], F16, tag="gproj2_tm")
            smat_s = apool.tile([128, 2, NPAD], F16, tag="smat")
            sidx_s = apool.tile([128, ecols], I16, tag="sidx")
            nodeB_sb = apool.tile([128, NW, 256], F16, tag="nodeB_sb")
            out_sb = apool.tile([8, NPAD], F32, tag="out_sb")

            nc.sync.dma_start(out=smat_s[:],
                              in_=smat[:].rearrange("p (kt n) -> p kt n", kt=2))
            nc.sync.dma_start(out=sidx_s[:], in_=sidx[:])

            # ============ phase LD: weight load + folds + emb + table ========
            with (tc.tile_pool(name="ld", bufs=1) as ld,
                  tc.tile_pool(name="lds", bufs=2) as lds):

                def load_f32r(name, dram_view, shape, pool=wp):
                    t = pool.tile(shape, F32R, tag=name)
                    nc.gpsimd.dma_start(out=t[:], in_=dram_view)
                    return t

                def load_f16(name, dram_view, shape, pool=wp):
                    t = pool.tile(shape, F16, tag=name)
                    nc.gpsimd.dma_start(out=t[:], in_=dram_view)
                    return t

                def load_bias(name, b, ntiles):
                    t = wp.tile([128, ntiles], F32, tag=name)
                    nc.sync.dma_start(out=t[:], in_=bview(b))
                    return t

                def load_row_f32(name, view, ncols, pool):
                    t = pool.tile([1, ncols], F32, tag=name)
                    nc.sync.dma_start(out=t[:], in_=view)
                    return t

                w1n_s = load_f32r("w1n", W["w1n"][:], [32, 512])
                xT_s = load_f32r("xT", xT[:], [32, NPAD], pool=ld)
                w2n_s = load_f32r("w2n", kview(W["w2n"]), [128, 4, 512])
                w3n_s = load_f32r("w3n", kview(W["w3n"]), [128, 4, 512])
                b1n_s = load_bias("b1n", W["b1n"], 4)
                b2n_s = load_bias("b2n", W["b2n"], 4)
                b3n_s = load_bias("b3n", W["b3n"], 4)

                # ---- 3-layer MLPs (feature-major) ----
                def mlp3_fm(x_s, n_tok, w1_s, b1_s, w2_s, b2_s, w3_s, b3_s, outs):
                    for ch in range((n_tok + 511) // 512):
                        lo = ch * 512
                        cw = min(512, n_tok - lo)
                        h1c = lds.tile([128, 4, 512], F32R, tag="mlp_h1")
                        for mt in range(4):
                            pt = pw()
                            nc.tensor.matmul(out=pt[:, :cw],
                                             lhsT=w1_s[:32, mt * 128:(mt + 1) * 128],
                                             rhs=x_s[:32, lo:lo + cw],
                                             start=True, stop=True)
                            nc.scalar.activation(h1c[:, mt, :cw], pt[:, :cw], RELU,
                                                 bias=b1_s[:, mt:mt + 1])
                        h2c = lds.tile([128, 4, 512], F32R, tag="mlp_h2")
                        for mt in range(4):
                            pt = pw()
                            for kt in range(4):
                                nc.tensor.matmul(
                                    out=pt[:, :cw],
                                    lhsT=w2_s[:, kt, mt * 128:(mt + 1) * 128],
                                    rhs=h1c[:, kt, :cw],
                                    start=(kt == 0), stop=(kt == 3))
                            nc.scalar.activation(h2c[:, mt, :cw], pt[:, :cw], RELU,
                                                 bias=b2_s[:, mt:mt + 1])
                        for mt in range(4):
                            pt = pw()
                            for kt in range(4):
                                nc.tensor.matmul(
                                    out=pt[:, :cw],
                                    lhsT=w3_s[:, kt, mt * 128:(mt + 1) * 128],
                                    rhs=h2c[:, kt, :cw],
                                    start=(kt == 0), stop=(kt == 3))
                            nc.vector.tensor_add(
                                outs[:, mt, lo:lo + cw], pt[:, :cw],
                                b3_s[:, mt:mt + 1].to_broadcast([128, cw]))

                # node embedding
                mlp3_fm(xT_s, NPAD, w1n_s, b1n_s, w2n_s, b2n_s, w3n_s, b3n_s, h_fm)


                w1g_s = load_f32r("w1g", W["w1g"][:], [32, 512], pool=ld)
                w2g_s = load_f32r("w2g", kview(W["w2g"]), [128, 4, 512], pool=ld)
                w3g_s = load_f32r("w3g", kview(W["w3g"]), [128, 4, 512], pool=ld)
                b1g_s = load_bias("b1g", W["b1g"], 4)
                b2g_s = load_bias("b2g", W["b2g"], 4)
                b3g_s = load_bias("b3g", W["b3g"], 4)

                w1e_s = load_f16("w1e", W["w1e"][:], [16, 256])
                w2e_s = load_f16("w2e", kview(W["w2e"]), [128, 2, 256])
                b1e_s = load_bias("b1e", W["b1e"], 2)
                b2e_s = load_bias("b2e", W["b2e"], 2)
                we3_s = load_f32r("we3", kview(W["we3"]), [128, 2, 256], pool=ld)

                # edge_mlp W1 splits
                w1ab_s = wp.tile([128, 4, 512], F32R, tag="w1ab")
                nc.gpsimd.dma_start(out=w1ab_s[:, :, 0:256],
                                    in_=rowslice(W["w1m"], 0, 512))
                nc.gpsimd.dma_start(out=w1ab_s[:, :, 256:512],
                                    in_=rowslice(W["w1m"], 512, 1024))
                w1c_s = load_f32r("w1c", rowslice(W["w1m"], 1024, 1280),
                                  [128, 2, 256], pool=ld)
                w1d_s = load_f32r("w1d", rowslice(W["w1m"], 1280, 1792),
                                  [128, 4, 256], pool=ld)
                w2m_s = load_f32r("w2m", kview(W["w2m"]), [128, 2, 256], pool=ld)

                wa_s = load_f32r("wa", rowslice(W["wn1"], 0, 512), [128, 4, 256])
                wb_s = load_f32r("wb", rowslice(W["wn1"], 512, 768), [128, 2, 256],
                                 pool=ld)
                wc_s = load_f32r("wc", rowslice(W["wn1"], 768, 1280), [128, 4, 256],
                                 pool=ld)
                bn1_s = load_bias("bn1", W["bn1"], 2)
                wn2_s = load_f32r("wn2", kview(W["wn2"]), [128, 2, 512], pool=ld)
                wh1_s = load_f32r("wh1", kview(W["wh1"]), [128, 4, 256], pool=ld)
                wh2_s = load_f32r("wh2", kview(W["wh2"]), [128, 2, 8])

                b1m_row = load_row_f32("b1m_row", W["b1m"][:].rearrange("(a c) -> a c", a=1),
                                       256, ld)
                bh1_row = load_row_f32("bh1_row", W["bh1"][:].rearrange("(a c) -> a c", a=1),
                                       256, ld)
                bh2_row = wp.tile([1, 8], F32R, tag="bh2_row")
                nc.gpsimd.dma_start(out=bh2_row[:],
                                    in_=W["bh2"][:].rearrange("(a c) -> a c", a=1))
                mrow_s = wp.tile([1, NPAD], F32R, tag="mrow")
                nc.gpsimd.dma_start(out=mrow_s[:], in_=mrow[:])
                be3_col = load_f32r("be3c",
                                    W["be3"][:].rearrange("(kt p a) -> p kt a", p=128, a=1),
                                    [128, 2, 1], pool=ld)
                b2m_col = load_f32r("b2mc",
                                    W["b2m"][:].rearrange("(kt p a) -> p kt a", p=128, a=1),
                                    [128, 2, 1], pool=ld)
                bn2_col = load_f32r("bn2c",
                                    W["bn2"][:].rearrange("(kt p a) -> p kt a", p=128, a=1),
                                    [128, 4, 1], pool=ld)

                # ---- weight folds ----
                def transpose_tiles(src, kt_n, mt_n, name):
                    dst = ld.tile([128, mt_n, kt_n * 128], F32R, tag=name)
                    for kt in range(kt_n):
                        for mt in range(mt_n):
                            pt = pk([128, 128], F32R)
                            nc.tensor.matmul(
                                pt[:], lhsT=src[:, kt, mt * 128:(mt + 1) * 128],
                                rhs=identr[:], is_transpose=True,
                                start=True, stop=True)
                            nc.vector.tensor_copy(
                                dst[:, mt, kt * 128:(kt + 1) * 128], pt[:])
                    return dst

                we3T = transpose_tiles(we3_s, 2, 2, "we3T")
                w2mT = transpose_tiles(w2m_s, 2, 2, "w2mT")
                wn2T = transpose_tiles(wn2_s, 2, 4, "wn2T")

                def fold_mm(lhsT_tiles, rhs_tiles, kt_n, mt_n, ncols, name, dt):
                    dst = wp.tile([128, mt_n, ncols], dt, tag=name)
                    for mt in range(mt_n):
                        pt = pk([128, ncols])
                        for kt in range(kt_n):
                            nc.tensor.matmul(
                                out=pt[:],
                                lhsT=lhsT_tiles[:, kt, mt * 128:(mt + 1) * 128],
                                rhs=rhs_tiles[:, kt, :],
                                start=(kt == 0), stop=(kt == kt_n - 1))
                        nc.scalar.activation(dst[:, mt, :], pt[:], COPY)
                    return dst

                w1cp = fold_mm(we3T, w1c_s, 2, 2, 256, "w1cp", F16)
                wbp = fold_mm(w2mT, wb_s, 2, 2, 256, "wbp", F32R)
                wfold = fold_mm(wn2T, wh1_s, 4, 2, 256, "wfold", F32R)

                def fold_vec(col_tiles, rhs_tiles, kt_n, ncols, add_row, name):
                    pt = pk([1, ncols])
                    for kt in range(kt_n):
                        nc.tensor.matmul(out=pt[:], lhsT=col_tiles[:, kt, :],
                                         rhs=rhs_tiles[:, kt, :],
                                         start=(kt == 0), stop=(kt == kt_n - 1))
                    dst = wp.tile([1, ncols], F32R, tag=name)
                    if add_row is not None:
                        tmp = lds.tile([1, ncols], F32, tag="fv_tmp")
                        nc.vector.tensor_add(tmp[:], pt[:], add_row[:])
                        nc.vector.tensor_copy(dst[:], tmp[:])
                    else:
                        nc.vector.tensor_copy(dst[:], pt[:])
                    return dst

                b1mp_row = fold_vec(be3_col, w1c_s, 2, 256, b1m_row, "b1mp")
                vb2_row = fold_vec(b2m_col, wb_s, 2, 256, None, "vb2")
                bfold_row = fold_vec(bn2_col, wh1_s, 4, 256, bh1_row, "bfold")
                b1mp_pad = wp.tile([1, 512], F32R, tag="b1mp_pad")
                nc.vector.tensor_copy(b1mp_pad[:], zero_f[0:1, :])
                nc.vector.tensor_copy(b1mp_pad[:, 0:256], b1mp_row[:])

                uT_s = load_f32r("uT", uT[:], [32, G], pool=ld)
                mlp3_fm(uT_s, G, w1g_s, b1g_s, w2g_s, b2g_s, w3g_s, b3g_s, g_fm)

                # gproj / gproj2 (token-major via LDW mode)
                def proj_tm(w_tiles, kt_n, dst):
                    for nt in range(G // 128):
                        pt = pk([128, 256])
                        for kt in range(kt_n):
                            nc.tensor.matmul(
                                out=pt[:],
                                lhsT=g_fm[:, kt, nt * 128:(nt + 1) * 128],
                                rhs=w_tiles[:, kt, :],
                                start=(kt == 0), stop=(kt == kt_n - 1))
                        nc.scalar.activation(dst[:, nt, :], pt[:], COPY)

                for _kt in range(2):
                    nc.vector.tensor_copy(gproj_pad[:, _kt, 256:512],
                                          zero_f[:, 0:256])
                gp_tmp = lds.tile([128, 2, 256], F16, tag="gp_tmp")
                proj_tm(w1d_s, 4, gp_tmp)
                nc.vector.tensor_copy(gproj_pad[:, :, 0:256], gp_tmp[:])
                proj_tm(wc_s, 4, gproj2_tm)

                # nodeA|nodeB table + AllGather
                for nt in range(NW):
                    pt = pw()
                    for kt in range(4):
                        nc.tensor.matmul(out=pt[:],
                                         lhsT=h_fm[:, kt, nt * 128:(nt + 1) * 128],
                                         rhs=w1ab_s[:, kt, :],
                                         start=(kt == 0), stop=False)
                    for gkt in range(2):
                        nc.tensor.matmul(out=pt[:],
                                         lhsT=smat_s[:, gkt, nt * 128:(nt + 1) * 128],
                                         rhs=gproj_pad[:, gkt, :],
                                         start=False, stop=False)
                    nc.tensor.matmul(out=pt[:], lhsT=ones_row[:, 0:128],
                                     rhs=b1mp_pad[:], start=False, stop=True)
                    ev = lds.tile([128, 512], F16, tag="nab_ev")
                    nc.scalar.activation(ev[:], pt[:], COPY)
                    nc.vector.tensor_copy(nodeB_sb[:, nt, :], ev[:, 256:512])
                    nc.sync.dma_start(out=ag_in[nt * 128:(nt + 1) * 128, :],
                                      in_=ev[:, 0:256])

                nc.gpsimd.collective_compute(
                    "AllGather", mybir.AluOpType.bypass,
                    replica_groups=[list(range(CORES))],
                    ins=[ag_in.ap().opt()], outs=[ag_out.ap().opt()],
                )

            if phases == 1:
                dbg = apool.tile([8, 256], F16, tag="dbg")
                nc.sync.dma_start(out=dbg[:], in_=ag_out[0:8, :])
                nc.vector.tensor_copy(out_sb[:8, 0:256], dbg[:])
                nc.sync.dma_start(out=out[:, 0:256], in_=out_sb[:8, 0:256])

            # ============ phase ES: edge stage ============
            with tc.tile_pool(name="es", bufs=2) as es:
                if phases < 2:
                    raise StopPhases

                tile_base = 0
                for w in range(NW):
                    tw = win_tiles[w]
                    e_lo = tile_base * 128
                    n_e = tw * 128
                    ea_w = es.tile([16, TWMAX * 128], F16, tag="ea_w")
                    nc.sync.dma_start(out=ea_w[:16, :n_e],
                                      in_=eaT[:, e_lo:e_lo + n_e])
                    m_w = es.tile([128, TWMAX, 128], F16, tag="m_w")
                    nc.sync.dma_start(
                        out=m_w[:, :tw, :],
                        in_=mmat[tile_base:tile_base + tw].rearrange("t p m -> p t m"))
                    gA = es.tile([128, TWMAX, 256], F16, tag="gA")
                    nc.gpsimd.dma_gather(
                        out_ap=gA[:, :tw, :], in_ap=ag_out[:, 0:256],
                        idxs_ap=sidx_s[:, e_lo // 16:(e_lo + n_e) // 16],
                        num_idxs=n_e, num_idxs_reg=n_e, elem_size=256,
                        elem_step=512, single_packet=False)
                    gB = es.tile([128, TWMAX, 256], F16, tag="gB")
                    nc.gpsimd.dma_gather(
                        out_ap=gB[:, :tw, :], in_ap=ag_out[:, 256:512],
                        idxs_ap=didx_s[:, e_lo // 16:(e_lo + n_e) // 16],
                        num_idxs=n_e, num_idxs_reg=n_e, elem_size=256,
                        elem_step=512, single_packet=False)

                    agg_ps = ps_agg.tile([128, 256], F32, tag="agg")
                    for ch in range((n_e + 511) // 512):
                        lo = ch * 512
                        cw = min(512, n_e - lo)
                        a1 = es.tile([128, 2, 512], F16, tag="ea_a1")
                        for mt in range(2):
                            pt = pw()
                            nc.tensor.matmul(
                                out=pt[:, :cw],
                                lhsT=w1e_s[:16, mt * 128:(mt + 1) * 128],
                                rhs=ea_w[:16, lo:lo + cw],
                                start=True, stop=True)
                            nc.scalar.activation(a1[:, mt, :cw], pt[:, :cw], RELU,
                                                 bias=b1e_s[:, mt:mt + 1])
                        a2 = es.tile([128, 2, 512], F16, tag="ea_a2")
                        for mt in range(2):
                            pt = pw()
                            for kt in range(2):
                                nc.tensor.matmul(
                                    out=pt[:, :cw],
                                    lhsT=w2e_s[:, kt, mt * 128:(mt + 1) * 128],
                                    rhs=a1[:, kt, :cw],
                                    start=(kt == 0), stop=(kt == 1))
                            nc.scalar.activation(a2[:, mt, :cw], pt[:, :cw], RELU,
                                                 bias=b2e_s[:, mt:mt + 1])
                        for tt in range(cw // 128):
                            gtile = (lo + tt * 128) // 128
                            pt = pk([128, 256])
                            for kt in range(2):
                                nc.tensor.matmul(
                                    out=pt[:],
                                    lhsT=a2[:, kt, tt * 128:(tt + 1) * 128],
                                    rhs=w1cp[:, kt, :],
                                    start=(kt == 0), stop=False)
                            nc.tensor.matmul(out=pt[:], lhsT=ident16[:],
                                             rhs=gA[:, gtile, :],
                                             start=False, stop=False)
                            nc.tensor.matmul(out=pt[:], lhsT=ident16[:],
                                             rhs=gB[:, gtile, :],
                                             start=False, stop=True)
                            a_e = es.tile([128, 256], F16, tag="a_e")
                            nc.scalar.activation(a_e[:], pt[:], RELU)
                            nc.tensor.matmul(out=agg_ps[:], lhsT=m_w[:, gtile, :],
                                             rhs=a_e[:],
                                             start=(gtile == 0),
                                             stop=(gtile == tw - 1))
                    agg_tm = es.tile([128, 256], F16, tag="agg_tm")
                    nc.scalar.activation(agg_tm[:], agg_ps[:], COPY)
                    for kt in range(2):
                        pt = pk([128, 128], F16)
                        nc.tensor.matmul(pt[:],
                                         lhsT=agg_tm[:, kt * 128:(kt + 1) * 128],
                                         rhs=ident16[:], is_transpose=True,
                                         start=True, stop=True)
                        nc.vector.tensor_copy(agg_fm[:, kt, w * 128:(w + 1) * 128],
                                              pt[:])
                    tile_base += tw

            if phases == 2:
                nc.vector.tensor_copy(out_sb[:8, :], agg_fm[0:8, 0, :])
                nc.sync.dma_start(out=out[:], in_=out_sb[:])

            # ============ phase NS: node stage ============
            with tc.tile_pool(name="ns", bufs=2) as ns:
                if phases < 3:
                    raise StopPhases

                for ch in range((NPAD + 511) // 512):
                    lo = ch * 512
                    cw = min(512, NPAD - lo)
                    a_n = ns.tile([128, 2, 512], F32R, tag="a_n")
                    for mt in range(2):
                        pt = pw()
                        for kt in range(4):
                            nc.tensor.matmul(
                                out=pt[:, :cw],
                                lhsT=wa_s[:, kt, mt * 128:(mt + 1) * 128],
                                rhs=h_fm[:, kt, lo:lo + cw],
                                start=(kt == 0), stop=False)
                        for kt in range(2):
                            nc.tensor.matmul(
                                out=pt[:, :cw],
                                lhsT=wbp[:, kt, mt * 128:(mt + 1) * 128],
                                rhs=agg_fm[:, kt, lo:lo + cw],
                                start=False, stop=False)
                        for kt in range(2):
                            nc.tensor.matmul(
                                out=pt[:, :cw],
                                lhsT=gproj2_tm[:, kt, mt * 128:(mt + 1) * 128],
                                rhs=smat_s[:, kt, lo:lo + cw],
                                start=False, stop=False)
                        nc.tensor.matmul(
                            out=pt[:, :cw],
                            lhsT=vb2_row[:, mt * 128:(mt + 1) * 128],
                            rhs=mrow_s[:, lo:lo + cw],
                            start=False, stop=True)
                        nc.scalar.activation(a_n[:, mt, :cw], pt[:, :cw], RELU,
                                             bias=bn1_s[:, mt:mt + 1])
                    z = ns.tile([128, 2, 512], F32R, tag="z")
                    for mt in range(2):
                        pt = pw()
                        for kt in range(2):
                            nc.tensor.matmul(
                                out=pt[:, :cw],
                                lhsT=wfold[:, kt, mt * 128:(mt + 1) * 128],
                                rhs=a_n[:, kt, :cw],
                                start=(kt == 0), stop=False)
                        nc.tensor.matmul(
                            out=pt[:, :cw],
                            lhsT=bfold_row[:, mt * 128:(mt + 1) * 128],
                            rhs=ones_row[:, :cw], start=False, stop=True)
                        nc.scalar.activation(z[:, mt, :cw], pt[:, :cw], RELU)
                    pt = pk([8, 512])
                    for kt in range(2):
                        nc.tensor.matmul(out=pt[:8, :cw], lhsT=wh2_s[:, kt, :],
                                         rhs=z[:, kt, :cw],
                                         start=(kt == 0), stop=False)
                    nc.tensor.matmul(out=pt[:8, :cw], lhsT=bh2_row[:],
                                     rhs=ones_row[:, :cw], start=False, stop=True)
                    nc.scalar.activation(out_sb[:8, lo:lo + cw], pt[:8, :cw], TANH)
                nc.sync.dma_start(out=out[:], in_=out_sb[:])

    nc.finalize()
    if split_waits:
        _split_excess_waits(nc)
    return nc


_BUILD_CACHE = {}


def _prepare(inputs):
    """Host-side sharding & index construction."""
    x = np.asarray(inputs["x"], dtype=np.float32)
    edge_index = np.asarray(inputs["edge_index"])
    edge_attr = np.asarray(inputs["edge_attr"], dtype=np.float32)
    u = np.asarray(inputs["u"], dtype=np.float32)
    batch = np.asarray(inputs["batch"]).astype(np.int64)
    src = edge_index[0].astype(np.int64)
    dst = edge_index[1].astype(np.int64)

    deg = np.bincount(dst, minlength=N).astype(np.float64)
    inv_cnt = (1.0 / np.maximum(deg, 1.0)).astype(np.float32)
    has_edge = (deg > 0).astype(np.float32)

    row_of = ((np.arange(N) // NPC) * NPAD + (np.arange(N) % NPC)).astype(np.int64)

    per_core = []
    for c in range(CORES):
        nlo = c * NPC
        m = (dst >= nlo) & (dst < nlo + NPC)
        eid = np.nonzero(m)[0]
        dloc = dst[eid] - nlo
        order = np.argsort(dloc, kind="stable")
        eid = eid[order]
        dloc = dloc[order]
        wins = []
        for w in range(NW):
            wm = (dloc >= w * 128) & (dloc < (w + 1) * 128)
            wins.append(eid[wm])
        per_core.append(wins)

    win_tiles = np.zeros(NW, dtype=np.int64)
    for w in range(NW):
        for c in range(CORES):
            win_tiles[w] = max(win_tiles[w], (len(per_core[c][w]) + 127) // 128)
    win_tiles = np.maximum(win_tiles, 1)
    EPC = int(win_tiles.sum()) * 128

    def wpair(p):
        return [np.ascontiguousarray(np.asarray(a, np.float32)) for a in p]

    (W1n, B1n), (W2n, B2n), (W3n, B3n) = [wpair(p) for p in inputs["node_emb_params"]]
    (W1e, B1e), (W2e, B2e), (We3, Be3) = [wpair(p) for p in inputs["edge_emb_params"]]
    (W1g, B1g), (W2g, B2g), (W3g, B3g) = [wpair(p) for p in inputs["glob_emb_params"]]
    (W1m, B1m), (W2m, B2m) = [wpair(p) for p in inputs["edge_mlp_params"]]
    (Wn1, Bn1), (Wn2, Bn2) = [wpair(p) for p in inputs["node_mlp_params"]]
    (Wh1, Bh1), (Wh2, Bh2) = [wpair(p) for p in inputs["head_params"]]

    shared = {
        "uT": np.ascontiguousarray(u.T), "w1n": W1n, "b1n": B1n, "w2n": W2n,
        "b2n": B2n, "w3n": W3n, "b3n": B3n, "w1e": W1e, "b1e": B1e,
        "w2e": W2e, "b2e": B2e, "we3": We3, "be3": Be3, "w1g": W1g,
        "b1g": B1g, "w2g": W2g, "b2g": B2g, "w3g": W3g, "b3g": B3g,
        "w1m": W1m, "b1m": B1m, "w2m": W2m, "b2m": B2m, "wn1": Wn1,
        "bn1": Bn1, "wn2": Wn2, "bn2": Bn2, "wh1": Wh1, "bh1": Bh1,
        "wh2": Wh2, "bh2": Bh2,
    }

    in_maps = []
    for c in range(CORES):
        nlo = c * NPC
        xT_c = np.zeros((32, NPAD), np.float32)
        xT_c[:, :NPC] = x[nlo:nlo + NPC].T
        gids = batch[nlo:nlo + NPC]
        s_l = np.zeros((128, 2, NPAD), np.float16)
        s_l[gids % 128, gids // 128, np.arange(NPC)] = 1.0
        mrow_c = np.zeros((1, NPAD), np.float32)
        mrow_c[0, :NPC] = has_edge[nlo:nlo + NPC]

        eaT_c = np.zeros((16, EPC), np.float16)
        sidx_c = np.zeros(EPC, np.int16)
        mmat_c = np.zeros((EPC // 128, 128, 128), np.float16)
        mmatT_c = np.zeros((EPC // 128, 128, 128), np.float16)
        off = 0
        for w in range(NW):
            eids = per_core[c][w]
            ne = len(eids)
            if ne:
                eaT_c[:, off:off + ne] = edge_attr[eids].T
                sidx_c[off:off + ne] = row_of[src[eids]]
                pos = np.arange(ne)
                dl = dst[eids] - nlo - w * 128
                mmat_c[off // 128 + pos // 128, pos % 128, dl] = \
                    inv_cnt[dst[eids]]
                mmatT_c[off // 128 + pos // 128, dl, pos % 128] = 1.0
            off += int(win_tiles[w]) * 128

        im = dict(shared)
        im.update({
            "xT": xT_c, "eaT": eaT_c,
            "sidx": _wrap_idx(sidx_c),
            "mmat": mmat_c, "mmatT": mmatT_c,
            "smat": s_l.reshape(128, 2 * NPAD),
            "mrow": mrow_c,
        })
        in_maps.append(im)
    return win_tiles, in_maps


def kernel(**inputs) -> np.ndarray:
    win_tiles, in_maps = _prepare(inputs)
    key = tuple(int(t) for t in win_tiles)
    if key not in _BUILD_CACHE:
        _BUILD_CACHE[key] = _build_graph(win_tiles)
    nc = _BUILD_CACHE[key]
    res = run_bass_kernel_spmd(nc, in_maps, core_ids=list(range(CORES)))
    out = np.empty((N, 8), np.float32)
    for c in range(CORES):
        out[c * NPC:(c + 1) * NPC] = res.results[c]["out"][:, :NPC].T
    return out
